# revision 1
# baseline (speedup 1.0000x reference)
"""Trainium2 Bass kernel for pre-LN multi-head self-attention.

Module: y = LN(x); qkv = y @ w_qkv; attention(8 heads, dh=64); out = ao @ w_out
Shapes: x [4, 2048, 512], w_qkv [512, 1536], w_out [512, 512], fp32.

Sharding (8 cores): core c -> batch b = c//2, head-group g = c%2 (4 heads).
Each core computes LN + QKV (its head slice) + attention + a partial output
projection (its heads' rows of w_out); the host sums the two partials per batch.

Per-core dataflow (transpose-free except one PE transpose of y):
  LN in natural [tok, d] layout (bn_stats) -> PE-transpose y -> yT [d, tok]
  Q^T, K^T = w^T @ yT   (features on partitions -- natural lhsT layout)
  V natural [tok, feat] with a fused ones-column so attn@V also accumulates
  the softmax denominator (row 64 of the PSUM accumulator).
  scoresT [k, q] = K^T.T @ Q^T per 128-k-token block; exp on ACT (no max
  subtraction needed: fp32, well-scaled inputs); attn@V accumulates over
  k-blocks in PSUM; per-head 1/sumexp broadcast via a DRAM roundtrip DMA;
  output projection consumes aoT directly as lhsT.
ln_scale/ln_bias are folded into w_qkv on the host (w_eff = scale*W,
bias_row = bias@W added per-feature on device), so the device LN is pure
normalize.  Matmul operands are bf16 (PSUM accumulation stays fp32); the
second matmul of each same-weight pair sets ldweights=False to skip the
redundant PE weight reload.  Stage D runs a depth-2 software pipeline
(scores+exp for item i+2 issue before attn@V of item i) so the in-order
PE never stalls on the ACT exp; the output projection is interleaved
per q-half, and the final unit normalizes in 128-token chunks so the
last projection tiles pipeline with it.
"""

import sys

if "/opt/trn_rl_repo" not in sys.path:
    sys.path.insert(0, "/opt/trn_rl_repo")

from contextlib import ExitStack

import numpy as np

import concourse.bass as bass
import concourse.tile as tile
from concourse.masks import make_identity
from concourse import bacc, mybir
from concourse.bass_utils import run_bass_kernel_spmd

B, N, D = 4, 2048, 512
H, DH = 8, 64
HPC = 4                 # heads per core
FPC = HPC * DH          # 256 features per core
P = 128
NT = N // P             # 16 token tiles
DT = D // P             # 4 d tiles
NQ = N // 512           # 4 q-blocks of 512
EPS = 1e-6
SCALE = DH ** -0.5
F32 = mybir.dt.float32
F32R = mybir.dt.float32r
BF16 = mybir.dt.bfloat16
ALU = mybir.AluOpType
AFT = mybir.ActivationFunctionType




def build_kernel():
    nc = bacc.Bacc("TRN2", target_bir_lowering=False, debug=False)
    xb = nc.dram_tensor("xb", [N, D], F32, kind="ExternalInput").ap()
    wq = nc.dram_tensor("wq", [D, FPC], BF16, kind="ExternalInput").ap()
    wk = nc.dram_tensor("wk", [D, FPC], BF16, kind="ExternalInput").ap()
    wv = nc.dram_tensor("wv", [D, FPC], BF16, kind="ExternalInput").ap()
    wo = nc.dram_tensor("wo", [FPC, D], BF16, kind="ExternalInput").ap()
    bq = nc.dram_tensor("bq", [FPC], F32, kind="ExternalInput").ap()
    bk = nc.dram_tensor("bk", [FPC], F32, kind="ExternalInput").ap()
    bv = nc.dram_tensor("bv", [FPC], F32, kind="ExternalInput").ap()
    out = nc.dram_tensor("out", [N, D], F32, kind="ExternalOutput").ap()

    with tile.TileContext(nc, pool_alloc_mode="queue") as tc, ExitStack() as ctx:
        consts = ctx.enter_context(tc.tile_pool(name="consts", bufs=1))
        big = ctx.enter_context(tc.tile_pool(name="big", bufs=1))
        dram = ctx.enter_context(tc.tile_pool(name="dram", bufs=2, space="DRAM"))

        identity = consts.tile([P, P], BF16)
        make_identity(nc, identity)
        eps_t = consts.tile([P, 1], F32)
        nc.vector.memset(eps_t, EPS)

        yT = [big.tile([P, N], BF16, tag=f"yT{j}", name=f"yT{j}") for j in range(DT)]
        qT = [big.tile([P, N], BF16, tag=f"qT{j}", name=f"qT{j}") for j in range(2)]
        kT = [big.tile([P, N], BF16, tag=f"kT{j}", name=f"kT{j}") for j in range(2)]
        aoT = [big.tile([P, N], BF16, tag=f"aoT{j}", name=f"aoT{j}") for j in range(2)]
        v_sb = big.tile([P, NT, HPC, DH + 1], BF16)
        ones_col = consts.tile([P, 1], F32)
        nc.vector.memset(ones_col, 1.0)
        nc.vector.tensor_copy(
            v_sb[:, :, :, DH : DH + 1],
            ones_col[:, 0:1].to_broadcast((P, NT, HPC, 1)),
        )

        # ---- Stage A+B: LayerNorm + transpose y -> yT ----
        with tc.tile_pool(name="ln", bufs=3) as ln, tc.tile_pool(
            name="tp_psum", bufs=4, space="PSUM"
        ) as tpp:
            for ig in range(NT // 4):  # groups of 4 token tiles
                y_ts = []
                for ii in range(4):
                    i = ig * 4 + ii
                    x_t = ln.tile([P, D], F32, tag="x")
                    nc.sync.dma_start(out=x_t, in_=xb[i * P : (i + 1) * P, :])
                    stats = ln.tile([P, 6], F32, tag="stats")
                    nc.vector.bn_stats(out=stats, in_=x_t)
                    mv = ln.tile([P, 2], F32, tag="mv")
                    nc.vector.bn_aggr(out=mv, in_=stats)
                    std = ln.tile([P, 1], F32, tag="std")
                    nc.scalar.activation(
                        out=std, in_=mv[:, 1:2], func=AFT.Sqrt, bias=eps_t[:, 0:1]
                    )
                    rstd = ln.tile([P, 1], F32, tag="rstd")
                    nc.vector.reciprocal(out=rstd, in_=std)
                    y_t = ln.tile([P, D], BF16, tag="y", bufs=6)
                    nc.vector.tensor_scalar(
                        out=y_t,
                        in0=x_t,
                        scalar1=mv[:, 0:1],
                        scalar2=rstd[:, 0:1],
                        op0=ALU.subtract,
                        op1=ALU.mult,
                    )
                    y_ts.append(y_t)
                for j in range(DT):
                    pt = tpp.tile([P, 512], BF16, tag="tp")
                    for ii in range(4):
                        nc.tensor.transpose(
                            pt[:, ii * P : (ii + 1) * P],
                            y_ts[ii][:, j * P : (j + 1) * P],
                            identity,
                        )
                    nc.scalar.activation(
                        out=yT[j][:, ig * 512 : (ig + 1) * 512],
                        in_=pt,
                        func=AFT.Copy,
                    )

        # weights: [d, f] -> sbuf [p, dt, f]
        w_q_sb = consts.tile([P, DT, FPC], BF16)
        nc.sync.dma_start(out=w_q_sb, in_=wq.rearrange("(t p) f -> p t f", p=P))
        w_k_sb = consts.tile([P, DT, FPC], BF16)
        nc.sync.dma_start(out=w_k_sb, in_=wk.rearrange("(t p) f -> p t f", p=P))
        w_v_sb = consts.tile([P, DT, FPC], BF16)
        nc.sync.dma_start(out=w_v_sb, in_=wv.rearrange("(t p) f -> p t f", p=P))
        w_o_sb = consts.tile([P, 2, D], BF16)
        nc.sync.dma_start(out=w_o_sb, in_=wo.rearrange("(t p) f -> p t f", p=P))
        bq_sb = consts.tile([P, 2], F32)
        nc.sync.dma_start(out=bq_sb, in_=bq.rearrange("(t p) -> p t", p=P))
        bk_sb = consts.tile([P, 2], F32)
        nc.sync.dma_start(out=bk_sb, in_=bk.rearrange("(t p) -> p t", p=P))
        bv_b = consts.tile([P, FPC], F32)
        bv_bcast = bass.AP(tensor=bv.tensor, offset=bv.offset, ap=[[0, P]] + list(bv.ap))
        nc.sync.dma_start(out=bv_b, in_=bv_bcast)

        # ---- Stage C: QKV projections ----
        with tc.tile_pool(name="c_psum", bufs=1, space="PSUM") as cpp, tc.tile_pool(
            name="v_psum", bufs=2, space="PSUM"
        ) as vpp:
            # j=0 groups first so heads 0/1 attention can start early; each
            # weight tile is loaded once and reused across the 4 token groups
            def qk_group(w_sb, b_sb, dstT, j):
                pss = [
                    cpp.tile([P, 512], F32, tag=f"qk{nt}", name=f"qk{nt}_{j}")
                    for nt in range(NQ)
                ]
                for dt in range(DT):
                    for nt in range(NQ):
                        mm = nc.tensor.matmul(
                            pss[nt],
                            lhsT=(w_sb[:, dt, j * P : (j + 1) * P]),
                            rhs=(yT[dt][:, nt * 512 : (nt + 1) * 512]),
                            start=(dt == 0),
                            stop=(dt == DT - 1),
                        )
                        if nt > 0:
                            mm.ins.ldweights = False
                for nt in range(NQ):
                    nc.scalar.activation(
                        out=dstT[j][:, nt * 512 : (nt + 1) * 512],
                        in_=pss[nt],
                        func=AFT.Identity,
                        bias=b_sb[:, j : j + 1],
                    )

            def v_group(irange):
                for i in irange:
                    ps = vpp.tile([P, FPC], F32, tag="v", name=f"v{i}")
                    for dt in range(DT):
                        nc.tensor.matmul(
                            ps,
                            lhsT=(yT[dt][:, i * P : (i + 1) * P]),
                            rhs=(w_v_sb[:, dt, :]),
                            start=(dt == 0),
                            stop=(dt == DT - 1),
                        )
                    nc.vector.tensor_tensor(
                        out=v_sb[:, i, :, 0:DH],
                        in0=ps.rearrange("p (h d) -> p h d", h=HPC),
                        in1=bv_b.rearrange("p (h d) -> p h d", h=HPC),
                        op=ALU.add,
                    )

            qk_group(w_k_sb, bk_sb, kT, 0)
            qk_group(w_q_sb, bq_sb, qT, 0)
            v_group(range(0, 8))
            qk_group(w_k_sb, bk_sb, kT, 1)
            qk_group(w_q_sb, bq_sb, qT, 1)
            v_group(range(8, NT))

        # ---- Stage D: attention, units of (q-half, head) pipelined ----
        QH = 1024
        with tc.tile_pool(name="sc_psum", bufs=2, space="PSUM") as scp, tc.tile_pool(
            name="ao_psum", bufs=1, space="PSUM"
        ) as aop, tc.tile_pool(name="exp_sb", bufs=6) as exps, tc.tile_pool(
            name="nrm", bufs=3
        ) as nrm, tc.tile_pool(
            name="o_psum", bufs=2, space="PSUM"
        ) as opp, tc.tile_pool(name="o_sb", bufs=3) as osb:
            items = [
                (qh, h, kb) for qh in range(2) for h in range(HPC) for kb in range(NT)
            ]
            ex_tiles = {}
            ao_tiles = {}

            def sc_exp(i):
                qh, h, kb = items[i]
                j, po = h // 2, (h % 2) * DH
                q0 = qh * QH
                sc = scp.tile([P, QH], F32, tag="sc", name=f"sc{i}")
                for c in range(2):
                    mm = nc.tensor.matmul(
                        sc[:, c * 512 : (c + 1) * 512],
                        lhsT=(kT[j][po : po + DH, kb * P : (kb + 1) * P]),
                        rhs=(qT[j][po : po + DH, q0 + c * 512 : q0 + (c + 1) * 512]),
                        start=True,
                        stop=True,
                    )
                    if c == 1:
                        mm.ins.ldweights = False
                ex = exps.tile([P, QH], BF16, tag="ex", name=f"ex{i}")
                nc.scalar.activation(out=ex, in_=sc, func=AFT.Exp, scale=SCALE)
                ex_tiles[i] = ex

            def attn_v(i):
                qh, h, kb = items[i]
                j, po = h // 2, (h % 2) * DH
                q0 = qh * QH
                if kb == 0:
                    ao_tiles[(qh, h)] = aop.tile(
                        [DH + 1, QH], F32, tag="ao", name=f"ao{qh}_{h}"
                    )
                ao_ps = ao_tiles[(qh, h)]
                ex = ex_tiles.pop(i)
                for c in range(2):
                    mm = nc.tensor.matmul(
                        ao_ps[:, c * 512 : (c + 1) * 512],
                        lhsT=(v_sb[:, kb, h, :]),
                        rhs=(ex[:, c * 512 : (c + 1) * 512]),
                        start=(kb == 0),
                        stop=(kb == NT - 1),
                    )
                    if c == 1:
                        mm.ins.ldweights = False
                if kb == NT - 1:
                    # evict unnormalized accumulator so the PSUM tile frees early
                    ao_sb = nrm.tile([DH + 1, QH], F32, tag="ao_sb", name=f"aosb{i}")
                    nc.vector.tensor_copy(ao_sb, ao_ps)
                    # the very last unit normalizes in 128-token chunks so the
                    # final output-projection tiles can pipeline with it
                    nchunk = 8 if i == len(items) - 1 else 1
                    cw = QH // nchunk
                    for ch in range(nchunk):
                        cs = ch * cw
                        recip = nrm.tile(
                            [1, QH], F32, tag="recip", name=f"rc{i}_{ch}", bufs=2
                        )
                        nc.vector.reciprocal(
                            out=recip[:, 0:cw],
                            in_=ao_sb[DH : DH + 1, cs : cs + cw],
                        )
                        scr = dram.tile([1, QH], F32, tag="scr", name=f"scr{i}{ch}")
                        nc.sync.dma_start(out=scr[:, 0:cw], in_=recip[:, 0:cw])
                        rb = nrm.tile(
                            [DH, QH], F32, tag="rb", name=f"rb{i}_{ch}", bufs=2
                        )
                        nc.sync.dma_start(
                            out=rb[:, 0:cw],
                            in_=scr[0:1, 0:cw].to_broadcast((DH, cw)),
                        )
                        nc.vector.tensor_tensor(
                            out=aoT[j][po : po + DH, q0 + cs : q0 + cs + cw],
                            in0=ao_sb[0:DH, cs : cs + cw],
                            in1=rb[:, 0:cw],
                            op=ALU.mult,
                        )
                        if nchunk > 1:
                            outproj_tile(NT // 2 + ch)

            def outproj_tile(mt):
                ps = opp.tile([P, D], F32, tag="o", name=f"o{mt}")
                for kt in range(2):
                    nc.tensor.matmul(
                        ps,
                        lhsT=(aoT[kt][:, mt * P : (mt + 1) * P]),
                        rhs=(w_o_sb[:, kt, :]),
                        start=(kt == 0),
                        stop=(kt == 1),
                    )
                ot = osb.tile([P, D], F32, tag="ot", name=f"ot{mt}")
                nc.vector.tensor_copy(ot, ps)
                nc.sync.dma_start(out=out[mt * P : (mt + 1) * P, :], in_=ot)

            DEPTH = 2
            for i in range(min(DEPTH, len(items))):
                sc_exp(i)
            for i in range(len(items)):
                if i + DEPTH < len(items):
                    sc_exp(i + DEPTH)
                attn_v(i)
                if items[i] == (0, HPC - 1, NT - 1):
                    for mt in range(NT // 2):
                        outproj_tile(mt)

    nc.compile()
    return nc


_NC_CACHE = None
_LAST_RESULT = None


def kernel(x, ln_scale, ln_bias, w_qkv, w_out):
    global _NC_CACHE, _LAST_RESULT
    if _NC_CACHE is None:
        _NC_CACHE = build_kernel()
    nc = _NC_CACHE

    import ml_dtypes

    x = np.asarray(x, np.float32)
    w_eff = (np.asarray(ln_scale, np.float32)[:, None] * np.asarray(w_qkv, np.float32))
    b_row = np.asarray(ln_bias, np.float32) @ np.asarray(w_qkv, np.float32)
    w_eff = w_eff.astype(ml_dtypes.bfloat16)
    w_out = np.asarray(w_out, np.float32).astype(ml_dtypes.bfloat16)

    in_maps = []
    for c in range(8):
        b, g = c // 2, c % 2
        s = slice(FPC * g, FPC * g + FPC)
        ks = slice(512 + FPC * g, 512 + FPC * g + FPC)
        vs = slice(1024 + FPC * g, 1024 + FPC * g + FPC)
        in_maps.append(
            {
                "xb": np.ascontiguousarray(x[b]),
                "wq": np.ascontiguousarray(w_eff[:, s]),
                "wk": np.ascontiguousarray(w_eff[:, ks]),
                "wv": np.ascontiguousarray(w_eff[:, vs]),
                "wo": np.ascontiguousarray(w_out[s, :]),
                "bq": np.ascontiguousarray(b_row[s]),
                "bk": np.ascontiguousarray(b_row[ks]),
                "bv": np.ascontiguousarray(b_row[vs]),
            }
        )
    res = run_bass_kernel_spmd(nc, in_maps, core_ids=list(range(8)))
    _LAST_RESULT = res
    outs = [res.results[c]["out"] for c in range(8)]
    return np.stack([outs[2 * b] + outs[2 * b + 1] for b in range(B)]).astype(
        np.float32
    )


if __name__ == "__main__":
    xs = np.random.randn(B, N, D).astype(np.float32)
    o = kernel(
        x=xs,
        ln_scale=np.ones(D, np.float32),
        ln_bias=np.zeros(D, np.float32),
        w_qkv=(np.random.randn(D, 3 * H * DH) / np.sqrt(D)).astype(np.float32),
        w_out=(np.random.randn(H * DH, D) / np.sqrt(H * DH)).astype(np.float32),
    )
    print(o.shape, o.dtype)



# revision 9
# speedup vs baseline: 1.0315x; 1.0315x over previous
"""Trainium2 Bass kernel for pre-LN multi-head self-attention.

Module: y = LN(x); qkv = y @ w_qkv; attention(8 heads, dh=64); out = ao @ w_out
Shapes: x [4, 2048, 512], w_qkv [512, 1536], w_out [512, 512], fp32.

Sharding (8 cores): core c -> batch b = c//2, head-group g = c%2 (4 heads).
Each core computes LN + QKV (its head slice) + attention + a partial output
projection (its heads' rows of w_out); the host sums the two partials per batch.

Design (v2, ACT-exp-stream centric):
  The softmax exp stream on the Scalar/ACT engine (16.8M elems/core at
  1 elem/cycle/lane @1.2GHz ~= 128us) is the hard floor; everything else is
  scheduled to keep that stream airtight and the PE clock warm (HAM K=8/8).
  - LN phase: 16-deep x-tile DMA lookahead; rstd = exp(-0.5*ln(var+eps)) so
    the whole kernel uses ONE ACT table set (natural_log_exp); y-affine on
    ACT, PSUM evictions on DVE; V-projection matmuls pipelined per token
    group to keep the PE busy during LN.
  - QK projections use [128,1024] PSUM accumulators from the same pool that
    later serves the score tiles; the j=1 head-pair projections are
    interleaved into early stage D so exps start right after j=0.
  - Stage D: depth-2 software pipeline (scores i+2 issue before attn@V i);
    single ao accumulator (eviction hides under the next unit's exp latency);
    softmax denominators (ones-column of V) broadcast across partitions via
    gpsimd.partition_broadcast and applied with a DVE divide -- no DRAM
    roundtrip, no 1-partition reciprocals.
  - Output projection tiles are spread one-per-item into PE slack; the last
    unit normalizes in 128-col chunks interleaved with the final tiles.
"""

import sys

if "/opt/trn_rl_repo" not in sys.path:
    sys.path.insert(0, "/opt/trn_rl_repo")

from contextlib import ExitStack

import numpy as np

import concourse.bass as bass
import concourse.tile as tile
from concourse.masks import make_identity
from concourse import bacc, mybir
from concourse.bass_utils import run_bass_kernel_spmd

B, N, D = 4, 2048, 512
H, DH = 8, 64
HPC = 4                 # heads per core
FPC = HPC * DH          # 256 features per core
P = 128
NT = N // P             # 16 token tiles
DT = D // P             # 4 d tiles
EPS = 1e-6
SCALE = DH ** -0.5
F32 = mybir.dt.float32
BF16 = mybir.dt.bfloat16
ALU = mybir.AluOpType
AFT = mybir.ActivationFunctionType
QH = 1024               # q-half width (stage D unit = (qh, h))


def build_kernel():
    nc = bacc.Bacc("TRN2", target_bir_lowering=False, debug=False)
    xb = nc.dram_tensor("xb", [N, D], F32, kind="ExternalInput").ap()
    wq = nc.dram_tensor("wq", [D, FPC], BF16, kind="ExternalInput").ap()
    wk = nc.dram_tensor("wk", [D, FPC], BF16, kind="ExternalInput").ap()
    wv = nc.dram_tensor("wv", [D, FPC], BF16, kind="ExternalInput").ap()
    wo = nc.dram_tensor("wo", [FPC, D], BF16, kind="ExternalInput").ap()
    bq = nc.dram_tensor("bq", [FPC], F32, kind="ExternalInput").ap()
    bk = nc.dram_tensor("bk", [FPC], F32, kind="ExternalInput").ap()
    bv = nc.dram_tensor("bv", [FPC], F32, kind="ExternalInput").ap()
    out = nc.dram_tensor("out", [N, D], F32, kind="ExternalOutput").ap()

    with tile.TileContext(nc, pool_alloc_mode="queue") as tc, ExitStack() as ctx:
        consts = ctx.enter_context(tc.tile_pool(name="consts", bufs=1))
        big = ctx.enter_context(tc.tile_pool(name="big", bufs=1))

        identity = consts.tile([P, P], BF16)
        make_identity(nc, identity)
        eps_t = consts.tile([P, 1], F32)
        nc.vector.memset(eps_t, EPS)
        # trigger the natural_log_exp ACT table load before anything depends
        # on the scalar engine
        warm = consts.tile([P, 1], F32)
        nc.scalar.activation(out=warm, in_=eps_t, func=AFT.Exp)

        yT = [big.tile([P, N], BF16, tag=f"yT{j}", name=f"yT{j}") for j in range(DT)]
        qT = [big.tile([P, N], BF16, tag=f"qT{j}", name=f"qT{j}") for j in range(2)]
        kT = [big.tile([P, N], BF16, tag=f"kT{j}", name=f"kT{j}") for j in range(2)]
        aoT = [big.tile([P, N], BF16, tag=f"aoT{j}", name=f"aoT{j}") for j in range(2)]
        v_sb = big.tile([P, NT, HPC, DH + 1], BF16)
        ones_col = consts.tile([P, 1], F32)
        nc.vector.memset(ones_col, 1.0)
        nc.vector.tensor_copy(
            v_sb[:, :, :, DH : DH + 1],
            ones_col[:, 0:1].to_broadcast((P, NT, HPC, 1)),
        )

        # ---- input + weight DMAs (x first; weights slot in between) ----
        xin = ctx.enter_context(tc.tile_pool(name="xin", bufs=16))
        x_ts = []
        for i in range(4):
            x_t = xin.tile([P, D], F32, tag="x", name=f"x{i}")
            nc.sync.dma_start(out=x_t, in_=xb[i * P : (i + 1) * P, :])
            x_ts.append(x_t)
        w_v_sb = consts.tile([P, DT, FPC], BF16)
        nc.sync.dma_start(out=w_v_sb, in_=wv.rearrange("(t p) f -> p t f", p=P))
        bv_b = consts.tile([P, FPC], F32)
        bv_bcast = bass.AP(tensor=bv.tensor, offset=bv.offset, ap=[[0, P]] + list(bv.ap))
        nc.sync.dma_start(out=bv_b, in_=bv_bcast)
        for i in range(4, 8):
            x_t = xin.tile([P, D], F32, tag="x", name=f"x{i}")
            nc.sync.dma_start(out=x_t, in_=xb[i * P : (i + 1) * P, :])
            x_ts.append(x_t)
        w_q_sb = consts.tile([P, DT, FPC], BF16)
        nc.sync.dma_start(out=w_q_sb, in_=wq.rearrange("(t p) f -> p t f", p=P))
        w_k_sb = consts.tile([P, DT, FPC], BF16)
        nc.sync.dma_start(out=w_k_sb, in_=wk.rearrange("(t p) f -> p t f", p=P))
        bq_sb = consts.tile([P, 2], F32)
        nc.sync.dma_start(out=bq_sb, in_=bq.rearrange("(t p) -> p t", p=P))
        bk_sb = consts.tile([P, 2], F32)
        nc.sync.dma_start(out=bk_sb, in_=bk.rearrange("(t p) -> p t", p=P))
        for i in range(8, NT):
            x_t = xin.tile([P, D], F32, tag="x", name=f"x{i}")
            nc.sync.dma_start(out=x_t, in_=xb[i * P : (i + 1) * P, :])
            x_ts.append(x_t)
        w_o_sb = consts.tile([P, 2, D], BF16)
        nc.sync.dma_start(out=w_o_sb, in_=wo.rearrange("(t p) f -> p t f", p=P))

        # ---- Phase A: LayerNorm + transpose + V projection, pipelined ----
        with tc.tile_pool(name="ln", bufs=4) as ln, tc.tile_pool(
            name="tp_psum", bufs=4, space="PSUM"
        ) as tpp, tc.tile_pool(name="v_psum", bufs=2, space="PSUM") as vpp:
            for ig in range(NT // 4):  # groups of 4 token tiles
                y_ts = []
                for ii in range(4):
                    i = ig * 4 + ii
                    x_t = x_ts[i]
                    stats = ln.tile([P, 6], F32, tag="stats")
                    nc.vector.bn_stats(out=stats, in_=x_t)
                    mv = ln.tile([P, 2], F32, tag="mv")
                    nc.vector.bn_aggr(out=mv, in_=stats)
                    # rstd = exp(-0.5 * ln(var + eps)): keeps ACT on the
                    # natural_log_exp table set for the whole kernel
                    lv = ln.tile([P, 1], F32, tag="lv")
                    nc.scalar.activation(
                        out=lv, in_=mv[:, 1:2], func=AFT.Ln, bias=eps_t[:, 0:1]
                    )
                    rstd = ln.tile([P, 1], F32, tag="rstd")
                    nc.scalar.activation(out=rstd, in_=lv, func=AFT.Exp, scale=-0.5)
                    nmr = ln.tile([P, 1], F32, tag="nmr")
                    nc.vector.tensor_scalar(
                        out=nmr,
                        in0=mv[:, 0:1],
                        scalar1=rstd[:, 0:1],
                        scalar2=-1.0,
                        op0=ALU.mult,
                        op1=ALU.mult,
                    )
                    y_t = ln.tile([P, D], BF16, tag="y", bufs=6)
                    nc.scalar.activation(
                        out=y_t,
                        in_=x_t,
                        func=AFT.Identity,
                        scale=rstd[:, 0:1],
                        bias=nmr[:, 0:1],
                    )
                    y_ts.append(y_t)
                for j in range(DT):
                    pt = tpp.tile([P, 512], BF16, tag="tp")
                    for ii in range(4):
                        nc.tensor.transpose(
                            pt[:, ii * P : (ii + 1) * P],
                            y_ts[ii][:, j * P : (j + 1) * P],
                            identity,
                        )
                    nc.vector.tensor_copy(
                        yT[j][:, ig * 512 : (ig + 1) * 512], pt
                    )
                # V projection for this group's 4 token tiles
                for ii in range(4):
                    i = ig * 4 + ii
                    ps = vpp.tile([P, FPC], F32, tag="v", name=f"v{i}")
                    for dt in range(DT):
                        nc.tensor.matmul(
                            ps,
                            lhsT=(yT[dt][:, i * P : (i + 1) * P]),
                            rhs=(w_v_sb[:, dt, :]),
                            start=(dt == 0),
                            stop=(dt == DT - 1),
                        )
                    nc.vector.tensor_tensor(
                        out=v_sb[:, i, :, 0:DH],
                        in0=ps.rearrange("p (h d) -> p h d", h=HPC),
                        in1=bv_b.rearrange("p (h d) -> p h d", h=HPC),
                        op=ALU.add,
                    )

        # ---- Phase B + D ----
        # bigp serves the j=0/j=1 QK accumulators and the stage-D score tiles
        bigp = ctx.enter_context(
            tc.tile_pool(name="bigp", bufs=2, space="PSUM")
        )

        def qk_half(w_sb, b_sb, dstT, j, half, on_act):
            ps = bigp.tile([P, QH], F32, tag="mm", name=f"qk{j}_{half}_{dstT is qT}")
            for dt in range(DT):
                for c in range(2):
                    mm = nc.tensor.matmul(
                        ps[:, c * 512 : (c + 1) * 512],
                        lhsT=(w_sb[:, dt, j * P : (j + 1) * P]),
                        rhs=(yT[dt][:, half * QH + c * 512 : half * QH + (c + 1) * 512]),
                        start=(dt == 0),
                        stop=(dt == DT - 1),
                    )
                    if c == 1:
                        mm.ins.ldweights = False
            cols = slice(half * QH, (half + 1) * QH)
            if on_act:
                nc.scalar.activation(
                    out=dstT[j][:, cols], in_=ps, func=AFT.Identity,
                    bias=b_sb[:, j : j + 1],
                )
            else:
                nc.vector.tensor_scalar(
                    out=dstT[j][:, cols], in0=ps, scalar1=b_sb[:, j : j + 1],
                    scalar2=None, op0=ALU.add,
                )

        # j=0 projections now (ACT evictions; exp stream hasn't started)
        for half in range(2):
            qk_half(w_k_sb, bk_sb, kT, 0, half, on_act=True)
        for half in range(2):
            qk_half(w_q_sb, bq_sb, qT, 0, half, on_act=True)

        # j=1 work in [128,512] chunks through the o_psum pool (idle until
        # the first out-projection), interleaved into early stage D
        j1_work = [
            (w, b, d, nt)
            for (w, b, d) in ((w_k_sb, bk_sb, kT), (w_q_sb, bq_sb, qT))
            for nt in range(4)
        ]

        # ---- Stage D ----
        with tc.tile_pool(name="ao_psum", bufs=1, space="PSUM") as aop, tc.tile_pool(
            name="exp_sb", bufs=6
        ) as exps, tc.tile_pool(name="nrm", bufs=3) as nrm, tc.tile_pool(
            name="o_psum", bufs=2, space="PSUM"
        ) as opp, tc.tile_pool(name="o_sb", bufs=3) as osb:
            items = [
                (qh, h, kb) for qh in range(2) for h in range(HPC) for kb in range(NT)
            ]
            ex_tiles = {}
            ao_tiles = {}

            def sc_exp(i):
                qh, h, kb = items[i]
                j, po = h // 2, (h % 2) * DH
                q0 = qh * QH
                sc = bigp.tile([P, QH], F32, tag="mm", name=f"sc{i}")
                for c in range(2):
                    mm = nc.tensor.matmul(
                        sc[:, c * 512 : (c + 1) * 512],
                        lhsT=(kT[j][po : po + DH, kb * P : (kb + 1) * P]),
                        rhs=(qT[j][po : po + DH, q0 + c * 512 : q0 + (c + 1) * 512]),
                        start=True,
                        stop=True,
                    )
                    if c == 1:
                        mm.ins.ldweights = False
                ex = exps.tile([P, QH], BF16, tag="ex", name=f"ex{i}")
                nc.scalar.activation(out=ex, in_=sc, func=AFT.Exp, scale=SCALE)
                ex_tiles[i] = ex

            def normalize(i, qh, h, j, po, q0, nchunk):
                # evict the unnormalized accumulator (frees the PSUM bank),
                # broadcast the denominator row across partitions on gpsimd,
                # then divide on DVE
                ao_ps = ao_tiles.pop((qh, h))
                ao_sb = nrm.tile([DH + 1, QH], F32, tag="ao_sb", name=f"aosb{i}")
                nc.vector.tensor_copy(ao_sb, ao_ps)
                cw = QH // nchunk
                for ch in range(nchunk):
                    cs = ch * cw
                    recip = nrm.tile([1, QH], F32, tag="rc", name=f"rc{i}_{ch}", bufs=2)
                    # NB: custom-DVE reciprocal_approx_fast mis-handles a
                    # non-zero base partition on HW; standard reciprocal is ok
                    nc.vector.reciprocal(
                        out=recip[:, 0:cw], in_=ao_sb[DH : DH + 1, cs : cs + cw]
                    )
                    rb = nrm.tile([DH, QH], F32, tag="rb", name=f"rb{i}_{ch}", bufs=2)
                    nc.gpsimd.partition_broadcast(
                        rb[:, 0:cw], recip[:, 0:cw], channels=DH
                    )
                    nc.vector.tensor_tensor(
                        out=aoT[j][po : po + DH, q0 + cs : q0 + cs + cw],
                        in0=ao_sb[0:DH, cs : cs + cw],
                        in1=rb[:, 0:cw],
                        op=ALU.mult,
                    )
                    if nchunk > 1:
                        outproj_tile(NT // 2 + ch)

            def attn_v(i):
                qh, h, kb = items[i]
                j, po = h // 2, (h % 2) * DH
                q0 = qh * QH
                if kb == 0:
                    ao_tiles[(qh, h)] = aop.tile(
                        [DH + 1, QH], F32, tag="ao", name=f"ao{qh}_{h}"
                    )
                ao_ps = ao_tiles[(qh, h)]
                ex = ex_tiles.pop(i)
                for c in range(2):
                    mm = nc.tensor.matmul(
                        ao_ps[:, c * 512 : (c + 1) * 512],
                        lhsT=(v_sb[:, kb, h, :]),
                        rhs=(ex[:, c * 512 : (c + 1) * 512]),
                        start=(kb == 0),
                        stop=(kb == NT - 1),
                    )
                    if c == 1:
                        mm.ins.ldweights = False
                if kb == NT - 1:
                    normalize(i, qh, h, j, po, q0, 8 if i == len(items) - 1 else 1)

            def outproj_tile(mt):
                ps = opp.tile([P, D], F32, tag="o", name=f"o{mt}")
                for kt in range(2):
                    nc.tensor.matmul(
                        ps,
                        lhsT=(aoT[kt][:, mt * P : (mt + 1) * P]),
                        rhs=(w_o_sb[:, kt, :]),
                        start=(kt == 0),
                        stop=(kt == 1),
                    )
                ot = osb.tile([P, D], F32, tag="ot", name=f"ot{mt}")
                nc.vector.tensor_copy(ot, ps)
                nc.sync.dma_start(out=out[mt * P : (mt + 1) * P, :], in_=ot)

            def qk1_chunk(w_sb, b_sb, dstT, nt):
                ps = opp.tile([P, 512], F32, tag="o", name=f"qk1_{dstT is qT}_{nt}")
                for dt in range(DT):
                    nc.tensor.matmul(
                        ps,
                        lhsT=(w_sb[:, dt, P : 2 * P]),
                        rhs=(yT[dt][:, nt * 512 : (nt + 1) * 512]),
                        start=(dt == 0),
                        stop=(dt == DT - 1),
                    )
                nc.vector.tensor_scalar(
                    out=dstT[1][:, nt * 512 : (nt + 1) * 512],
                    in0=ps, scalar1=b_sb[:, 1:2], scalar2=None, op0=ALU.add,
                )

            # item index -> extra PE work issued right after attn_v(i):
            # j=1 QK chunks early (heads 2/3 start at item 32), first
            # out-projection half spread after unit (0,3) completes
            post = {}
            for n, w in enumerate(j1_work):
                post[2 + 3 * n] = ("qk", w)
            for mt in range(NT // 2):
                post[68 + mt] = ("op", mt)

            DEPTH = 2
            for i in range(min(DEPTH, len(items))):
                sc_exp(i)
            for i in range(len(items)):
                if i + DEPTH < len(items):
                    sc_exp(i + DEPTH)
                attn_v(i)
                extra = post.get(i)
                if extra is not None:
                    if extra[0] == "qk":
                        qk1_chunk(*extra[1])
                    else:
                        outproj_tile(extra[1])

    nc.compile()
    return nc


_NC_CACHE = None
_LAST_RESULT = None


def kernel(x, ln_scale, ln_bias, w_qkv, w_out):
    global _NC_CACHE, _LAST_RESULT
    if _NC_CACHE is None:
        _NC_CACHE = build_kernel()
    nc = _NC_CACHE

    import ml_dtypes

    x = np.asarray(x, np.float32)
    w_eff = (np.asarray(ln_scale, np.float32)[:, None] * np.asarray(w_qkv, np.float32))
    b_row = np.asarray(ln_bias, np.float32) @ np.asarray(w_qkv, np.float32)
    w_eff = w_eff.astype(ml_dtypes.bfloat16)
    w_out = np.asarray(w_out, np.float32).astype(ml_dtypes.bfloat16)

    in_maps = []
    for c in range(8):
        b, g = c // 2, c % 2
        s = slice(FPC * g, FPC * g + FPC)
        ks = slice(512 + FPC * g, 512 + FPC * g + FPC)
        vs = slice(1024 + FPC * g, 1024 + FPC * g + FPC)
        in_maps.append(
            {
                "xb": np.ascontiguousarray(x[b]),
                "wq": np.ascontiguousarray(w_eff[:, s]),
                "wk": np.ascontiguousarray(w_eff[:, ks]),
                "wv": np.ascontiguousarray(w_eff[:, vs]),
                "wo": np.ascontiguousarray(w_out[s, :]),
                "bq": np.ascontiguousarray(b_row[s]),
                "bk": np.ascontiguousarray(b_row[ks]),
                "bv": np.ascontiguousarray(b_row[vs]),
            }
        )
    res = run_bass_kernel_spmd(nc, in_maps, core_ids=list(range(8)))
    _LAST_RESULT = res
    outs = [res.results[c]["out"] for c in range(8)]
    return np.stack([outs[2 * b] + outs[2 * b + 1] for b in range(B)]).astype(
        np.float32
    )


if __name__ == "__main__":
    xs = np.random.randn(B, N, D).astype(np.float32)
    o = kernel(
        x=xs,
        ln_scale=np.ones(D, np.float32),
        ln_bias=np.zeros(D, np.float32),
        w_qkv=(np.random.randn(D, 3 * H * DH) / np.sqrt(D)).astype(np.float32),
        w_out=(np.random.randn(H * DH, D) / np.sqrt(H * DH)).astype(np.float32),
    )
    print(o.shape, o.dtype)


# revision 16
# speedup vs baseline: 1.4685x; 1.4236x over previous
"""Trainium2 Bass kernel for pre-LN multi-head self-attention.

Module: y = LN(x); qkv = y @ w_qkv; attention(8 heads, dh=64); out = ao @ w_out
Shapes: x [4, 2048, 512], w_qkv [512, 1536], w_out [512, 512], fp32.

Sharding (8 cores): core c -> batch b = c//2, head-group g = c%2 (4 heads).
Each core computes LN + QKV (its head slice) + attention + a partial output
projection (its heads' rows of w_out); the host sums the two partials per batch.

Design (v2, ACT-exp-stream centric):
  The softmax exp stream on the Scalar/ACT engine (16.8M elems/core at
  1 elem/cycle/lane @1.2GHz ~= 128us) is the hard floor; everything else is
  scheduled to keep that stream airtight and the PE clock warm (HAM K=8/8).
  - LN phase: 16-deep x-tile DMA lookahead; rstd = exp(-0.5*ln(var+eps)) so
    the whole kernel uses ONE ACT table set (natural_log_exp); y-affine on
    ACT, PSUM evictions on DVE; V-projection matmuls pipelined per token
    group to keep the PE busy during LN.
  - QK projections use [128,1024] PSUM accumulators from the same pool that
    later serves the score tiles; the j=1 head-pair projections are
    interleaved into early stage D so exps start right after j=0.
  - Stage D: depth-2 software pipeline (scores i+2 issue before attn@V i);
    single ao accumulator (eviction hides under the next unit's exp latency);
    softmax denominators (ones-column of V) broadcast across partitions via
    gpsimd.partition_broadcast and applied with a DVE divide -- no DRAM
    roundtrip, no 1-partition reciprocals.
  - Output projection tiles are spread one-per-item into PE slack; the last
    unit normalizes in 128-col chunks interleaved with the final tiles.
"""

import sys

if "/opt/trn_rl_repo" not in sys.path:
    sys.path.insert(0, "/opt/trn_rl_repo")

from contextlib import ExitStack

import numpy as np

import concourse.bass as bass
import concourse.tile as tile
from concourse.masks import make_identity
from concourse import bacc, mybir
from concourse.bass_utils import run_bass_kernel_spmd

B, N, D = 4, 2048, 512
H, DH = 8, 64
HPC = 4                 # heads per core
FPC = HPC * DH          # 256 features per core
P = 128
NT = N // P             # 16 token tiles
DT = D // P             # 4 d tiles
EPS = 1e-6
SCALE = DH ** -0.5
F32 = mybir.dt.float32
BF16 = mybir.dt.bfloat16
ALU = mybir.AluOpType
AFT = mybir.ActivationFunctionType
QH = 1024               # q-half width (stage D unit = (qh, h))


def build_kernel():
    nc = bacc.Bacc("TRN2", target_bir_lowering=False, debug=False)
    xb = nc.dram_tensor("xb", [N, D], F32, kind="ExternalInput").ap()
    wq = nc.dram_tensor("wq", [D, FPC], BF16, kind="ExternalInput").ap()
    wk = nc.dram_tensor("wk", [D, FPC], BF16, kind="ExternalInput").ap()
    wv = nc.dram_tensor("wv", [D, FPC], BF16, kind="ExternalInput").ap()
    wo = nc.dram_tensor("wo", [FPC, D], BF16, kind="ExternalInput").ap()
    bq = nc.dram_tensor("bq", [FPC], F32, kind="ExternalInput").ap()
    bk = nc.dram_tensor("bk", [FPC], F32, kind="ExternalInput").ap()
    bv = nc.dram_tensor("bv", [FPC], F32, kind="ExternalInput").ap()
    out = nc.dram_tensor("out", [N, D], F32, kind="ExternalOutput").ap()

    with tile.TileContext(nc, pool_alloc_mode="queue") as tc, ExitStack() as ctx:
        consts = ctx.enter_context(tc.tile_pool(name="consts", bufs=1))
        big = ctx.enter_context(tc.tile_pool(name="big", bufs=1))

        identity = consts.tile([P, P], BF16)
        make_identity(nc, identity)
        eps_t = consts.tile([P, 1], F32)
        nc.vector.memset(eps_t, EPS)

        yT = [big.tile([P, N], BF16, tag=f"yT{j}", name=f"yT{j}") for j in range(DT)]
        qT = [big.tile([P, N], BF16, tag=f"qT{j}", name=f"qT{j}") for j in range(2)]
        kT = [big.tile([P, N], BF16, tag=f"kT{j}", name=f"kT{j}") for j in range(2)]
        aoT = [big.tile([P, N], BF16, tag=f"aoT{j}", name=f"aoT{j}") for j in range(2)]
        v_sb = big.tile([P, NT, HPC, DH + 1], BF16)
        ones_col = consts.tile([P, 1], F32)
        nc.vector.memset(ones_col, 1.0)
        nc.vector.tensor_copy(
            v_sb[:, :, :, DH : DH + 1],
            ones_col[:, 0:1].to_broadcast((P, NT, HPC, 1)),
        )

        # ---- input + weight DMAs: x in 4 batched group DMAs (one trigger
        # each -- the sync engine serializes triggers at ~600ns apiece)
        xin = ctx.enter_context(tc.tile_pool(name="xin", bufs=4))
        x_gs = []
        for ig in range(4):
            x_g = xin.tile([P, 4, D], F32, tag="xg", name=f"xg{ig}")
            nc.sync.dma_start(
                out=x_g,
                in_=xb[ig * 512 : (ig + 1) * 512, :].rearrange(
                    "(t p) d -> p t d", p=P
                ),
            )
            x_gs.append(x_g)
            if ig == 0:
                w_v_sb = consts.tile([P, DT, FPC], BF16)
                nc.sync.dma_start(
                    out=w_v_sb, in_=wv.rearrange("(t p) f -> p t f", p=P)
                )
                bv_b = consts.tile([P, FPC], F32)
                bv_bcast = bass.AP(
                    tensor=bv.tensor, offset=bv.offset, ap=[[0, P]] + list(bv.ap)
                )
                nc.sync.dma_start(out=bv_b, in_=bv_bcast)
            if ig == 1:
                w_q_sb = consts.tile([P, DT, FPC], BF16)
                nc.sync.dma_start(
                    out=w_q_sb, in_=wq.rearrange("(t p) f -> p t f", p=P)
                )
                w_k_sb = consts.tile([P, DT, FPC], BF16)
                nc.sync.dma_start(
                    out=w_k_sb, in_=wk.rearrange("(t p) f -> p t f", p=P)
                )
            if ig == 2:
                bq_sb = consts.tile([P, 2], F32)
                nc.sync.dma_start(out=bq_sb, in_=bq.rearrange("(t p) -> p t", p=P))
                bk_sb = consts.tile([P, 2], F32)
                nc.sync.dma_start(out=bk_sb, in_=bk.rearrange("(t p) -> p t", p=P))
                w_o_sb = consts.tile([P, 2, D], BF16)
                nc.sync.dma_start(
                    out=w_o_sb, in_=wo.rearrange("(t p) f -> p t f", p=P)
                )

        # ---- Phase A: LayerNorm + transpose + V projection, pipelined ----
        with tc.tile_pool(name="ln", bufs=4) as ln, tc.tile_pool(
            name="tp_psum", bufs=4, space="PSUM"
        ) as tpp, tc.tile_pool(name="v_psum", bufs=2, space="PSUM") as vpp:
            for ig in range(NT // 4):  # groups of 4 token tiles
                y_ts = []
                for ii in range(4):
                    i = ig * 4 + ii
                    x_t = x_gs[ig][:, ii, :]
                    stats = ln.tile([P, 6], F32, tag="stats")
                    nc.vector.bn_stats(out=stats, in_=x_t)
                    mv = ln.tile([P, 2], F32, tag="mv")
                    nc.vector.bn_aggr(out=mv, in_=stats)
                    std = ln.tile([P, 1], F32, tag="std")
                    nc.scalar.activation(
                        out=std, in_=mv[:, 1:2], func=AFT.Sqrt, bias=eps_t[:, 0:1]
                    )
                    rstd = ln.tile([P, 1], F32, tag="rstd")
                    nc.vector.reciprocal(out=rstd, in_=std)
                    nmr = ln.tile([P, 1], F32, tag="nmr")
                    nc.vector.tensor_scalar(
                        out=nmr,
                        in0=mv[:, 0:1],
                        scalar1=rstd[:, 0:1],
                        scalar2=-1.0,
                        op0=ALU.mult,
                        op1=ALU.mult,
                    )
                    y_t = ln.tile([P, D], BF16, tag="y", bufs=6)
                    nc.scalar.activation(
                        out=y_t,
                        in_=x_t,
                        func=AFT.Identity,
                        scale=rstd[:, 0:1],
                        bias=nmr[:, 0:1],
                    )
                    y_ts.append(y_t)
                for j in range(DT):
                    pt = tpp.tile([P, 512], BF16, tag="tp")
                    for ii in range(4):
                        nc.tensor.transpose(
                            pt[:, ii * P : (ii + 1) * P],
                            y_ts[ii][:, j * P : (j + 1) * P],
                            identity,
                        )
                    nc.vector.tensor_copy(
                        yT[j][:, ig * 512 : (ig + 1) * 512], pt
                    )
                # V projection for this group's 4 token tiles
                for ii in range(4):
                    i = ig * 4 + ii
                    ps = vpp.tile([P, FPC], F32, tag="v", name=f"v{i}")
                    for dt in range(DT):
                        nc.tensor.matmul(
                            ps,
                            lhsT=(yT[dt][:, i * P : (i + 1) * P]),
                            rhs=(w_v_sb[:, dt, :]),
                            start=(dt == 0),
                            stop=(dt == DT - 1),
                        )
                    nc.vector.tensor_tensor(
                        out=v_sb[:, i, :, 0:DH],
                        in0=ps.rearrange("p (h d) -> p h d", h=HPC),
                        in1=bv_b.rearrange("p (h d) -> p h d", h=HPC),
                        op=ALU.add,
                    )

        # ---- Phase B + D ----
        # bigp serves the j=0/j=1 QK accumulators and the stage-D score tiles
        bigp = ctx.enter_context(
            tc.tile_pool(name="bigp", bufs=2, space="PSUM")
        )

        def qk_half(w_sb, b_sb, dstT, j, half, on_act):
            ps = bigp.tile([P, QH], F32, tag="mm", name=f"qk{j}_{half}_{dstT is qT}")
            for dt in range(DT):
                for c in range(2):
                    mm = nc.tensor.matmul(
                        ps[:, c * 512 : (c + 1) * 512],
                        lhsT=(w_sb[:, dt, j * P : (j + 1) * P]),
                        rhs=(yT[dt][:, half * QH + c * 512 : half * QH + (c + 1) * 512]),
                        start=(dt == 0),
                        stop=(dt == DT - 1),
                    )
                    if c == 1:
                        mm.ins.ldweights = False
            cols = slice(half * QH, (half + 1) * QH)
            if on_act:
                nc.scalar.activation(
                    out=dstT[j][:, cols], in_=ps, func=AFT.Identity,
                    bias=b_sb[:, j : j + 1],
                )
            else:
                nc.vector.tensor_scalar(
                    out=dstT[j][:, cols], in0=ps, scalar1=b_sb[:, j : j + 1],
                    scalar2=None, op0=ALU.add,
                )

        # j=0 projections now (ACT evictions; exp stream hasn't started)
        for half in range(2):
            qk_half(w_k_sb, bk_sb, kT, 0, half, on_act=True)
        for half in range(2):
            qk_half(w_q_sb, bq_sb, qT, 0, half, on_act=True)

        # j=1 work in [128,512] chunks through the o_psum pool (idle until
        # the first out-projection), interleaved into early stage D
        j1_work = [
            (w, b, d, nt)
            for (w, b, d) in ((w_k_sb, bk_sb, kT), (w_q_sb, bq_sb, qT))
            for nt in range(4)
        ]

        # ---- Stage D ----
        with tc.tile_pool(name="ao_psum", bufs=1, space="PSUM") as aop, tc.tile_pool(
            name="exp_sb", bufs=6
        ) as exps, tc.tile_pool(name="nrm", bufs=3) as nrm, tc.tile_pool(
            name="o_psum", bufs=2, space="PSUM"
        ) as opp, tc.tile_pool(name="o_sb", bufs=3) as osb:
            items = [
                (qh, h, kb) for qh in range(2) for h in range(HPC) for kb in range(NT)
            ]
            ex_tiles = {}
            ao_tiles = {}

            def sc_exp(i):
                qh, h, kb = items[i]
                j, po = h // 2, (h % 2) * DH
                q0 = qh * QH
                sc = bigp.tile([P, QH], F32, tag="mm", name=f"sc{i}")
                for c in range(2):
                    mm = nc.tensor.matmul(
                        sc[:, c * 512 : (c + 1) * 512],
                        lhsT=(kT[j][po : po + DH, kb * P : (kb + 1) * P]),
                        rhs=(qT[j][po : po + DH, q0 + c * 512 : q0 + (c + 1) * 512]),
                        start=True,
                        stop=True,
                    )
                    if c == 1:
                        mm.ins.ldweights = False
                ex = exps.tile([P, QH], BF16, tag="ex", name=f"ex{i}")
                nc.scalar.activation(out=ex, in_=sc, func=AFT.Exp, scale=SCALE)
                ex_tiles[i] = ex

            def normalize(i, qh, h, j, po, q0, nchunk):
                # evict the unnormalized accumulator (frees the PSUM bank),
                # broadcast the denominator row across partitions on gpsimd,
                # then divide on DVE
                ao_ps = ao_tiles.pop((qh, h))
                ao_sb = nrm.tile([DH + 1, QH], F32, tag="ao_sb", name=f"aosb{i}")
                nc.vector.tensor_copy(ao_sb, ao_ps)
                cw = QH // nchunk
                # stage the denominator row onto partition 0: the fast
                # 1-cyc/elem custom-DVE reciprocal reads partition 0 only
                # (standard reciprocal is ~7 cyc/elem)
                dn = nrm.tile([1, QH], F32, tag="dn", name=f"dn{i}", bufs=2)
                nc.vector.tensor_copy(dn, ao_sb[DH : DH + 1, :])
                for ch in range(nchunk):
                    cs = ch * cw
                    recip = nrm.tile([1, QH], F32, tag="rc", name=f"rc{i}_{ch}", bufs=2)
                    nc.vector.reciprocal_approx_fast(
                        out=recip[:, 0:cw], in_=dn[0:1, cs : cs + cw]
                    )
                    rb = nrm.tile([DH, QH], F32, tag="rb", name=f"rb{i}_{ch}", bufs=2)
                    nc.gpsimd.partition_broadcast(
                        rb[:, 0:cw], recip[:, 0:cw], channels=DH
                    )
                    nc.vector.tensor_tensor(
                        out=aoT[j][po : po + DH, q0 + cs : q0 + cs + cw],
                        in0=ao_sb[0:DH, cs : cs + cw],
                        in1=rb[:, 0:cw],
                        op=ALU.mult,
                    )
                    if nchunk > 1:
                        outproj_tile(NT // 2 + ch)

            def attn_v(i):
                qh, h, kb = items[i]
                j, po = h // 2, (h % 2) * DH
                q0 = qh * QH
                if kb == 0:
                    ao_tiles[(qh, h)] = aop.tile(
                        [DH + 1, QH], F32, tag="ao", name=f"ao{qh}_{h}"
                    )
                ao_ps = ao_tiles[(qh, h)]
                ex = ex_tiles.pop(i)
                for c in range(2):
                    mm = nc.tensor.matmul(
                        ao_ps[:, c * 512 : (c + 1) * 512],
                        lhsT=(v_sb[:, kb, h, :]),
                        rhs=(ex[:, c * 512 : (c + 1) * 512]),
                        start=(kb == 0),
                        stop=(kb == NT - 1),
                    )
                    if c == 1:
                        mm.ins.ldweights = False
                if kb == NT - 1:
                    normalize(i, qh, h, j, po, q0, 8 if i == len(items) - 1 else 1)

            def outproj_tile(mt):
                ps = opp.tile([P, D], F32, tag="o", name=f"o{mt}")
                for kt in range(2):
                    nc.tensor.matmul(
                        ps,
                        lhsT=(aoT[kt][:, mt * P : (mt + 1) * P]),
                        rhs=(w_o_sb[:, kt, :]),
                        start=(kt == 0),
                        stop=(kt == 1),
                    )
                ot = osb.tile([P, D], F32, tag="ot", name=f"ot{mt}")
                nc.vector.tensor_copy(ot, ps)
                nc.sync.dma_start(out=out[mt * P : (mt + 1) * P, :], in_=ot)

            def qk1_chunk(w_sb, b_sb, dstT, nt):
                ps = opp.tile([P, 512], F32, tag="o", name=f"qk1_{dstT is qT}_{nt}")
                for dt in range(DT):
                    nc.tensor.matmul(
                        ps,
                        lhsT=(w_sb[:, dt, P : 2 * P]),
                        rhs=(yT[dt][:, nt * 512 : (nt + 1) * 512]),
                        start=(dt == 0),
                        stop=(dt == DT - 1),
                    )
                nc.vector.tensor_scalar(
                    out=dstT[1][:, nt * 512 : (nt + 1) * 512],
                    in0=ps, scalar1=b_sb[:, 1:2], scalar2=None, op0=ALU.add,
                )

            # item index -> extra PE work issued right after attn_v(i):
            # j=1 QK chunks early (heads 2/3 start at item 32), first
            # out-projection half spread after unit (0,3) completes
            post = {}
            for n, w in enumerate(j1_work):
                post[2 + 3 * n] = ("qk", w)
            for mt in range(NT // 2):
                post[76 + 2 * mt] = ("op", mt)

            DEPTH = 2
            for i in range(min(DEPTH, len(items))):
                sc_exp(i)
            for i in range(len(items)):
                if i + DEPTH < len(items):
                    sc_exp(i + DEPTH)
                attn_v(i)
                extra = post.get(i)
                if extra is not None:
                    if extra[0] == "qk":
                        qk1_chunk(*extra[1])
                    else:
                        outproj_tile(extra[1])

    nc.compile()
    return nc


_NC_CACHE = None
_LAST_RESULT = None


def kernel(x, ln_scale, ln_bias, w_qkv, w_out):
    global _NC_CACHE, _LAST_RESULT
    if _NC_CACHE is None:
        _NC_CACHE = build_kernel()
    nc = _NC_CACHE

    import ml_dtypes

    x = np.asarray(x, np.float32)
    w_eff = (np.asarray(ln_scale, np.float32)[:, None] * np.asarray(w_qkv, np.float32))
    b_row = np.asarray(ln_bias, np.float32) @ np.asarray(w_qkv, np.float32)
    w_eff = w_eff.astype(ml_dtypes.bfloat16)
    w_out = np.asarray(w_out, np.float32).astype(ml_dtypes.bfloat16)

    in_maps = []
    for c in range(8):
        b, g = c // 2, c % 2
        s = slice(FPC * g, FPC * g + FPC)
        ks = slice(512 + FPC * g, 512 + FPC * g + FPC)
        vs = slice(1024 + FPC * g, 1024 + FPC * g + FPC)
        in_maps.append(
            {
                "xb": np.ascontiguousarray(x[b]),
                "wq": np.ascontiguousarray(w_eff[:, s]),
                "wk": np.ascontiguousarray(w_eff[:, ks]),
                "wv": np.ascontiguousarray(w_eff[:, vs]),
                "wo": np.ascontiguousarray(w_out[s, :]),
                "bq": np.ascontiguousarray(b_row[s]),
                "bk": np.ascontiguousarray(b_row[ks]),
                "bv": np.ascontiguousarray(b_row[vs]),
            }
        )
    res = run_bass_kernel_spmd(nc, in_maps, core_ids=list(range(8)))
    _LAST_RESULT = res
    outs = [res.results[c]["out"] for c in range(8)]
    return np.stack([outs[2 * b] + outs[2 * b + 1] for b in range(B)]).astype(
        np.float32
    )


if __name__ == "__main__":
    xs = np.random.randn(B, N, D).astype(np.float32)
    o = kernel(
        x=xs,
        ln_scale=np.ones(D, np.float32),
        ln_bias=np.zeros(D, np.float32),
        w_qkv=(np.random.randn(D, 3 * H * DH) / np.sqrt(D)).astype(np.float32),
        w_out=(np.random.randn(H * DH, D) / np.sqrt(H * DH)).astype(np.float32),
    )
    print(o.shape, o.dtype)
